# revision 4
# baseline (speedup 1.0000x reference)
"""Bass/Tile TRN2 kernel for nn_Decoder_Transformer (B=2, S=1024, D=1024, H=16,
L=4, DFF=4096, 3 output heads) on 8 NeuronCores.

Sharding: balanced causal sequence-parallel ("zebra"). Core c serves batch
b=c//4 and owns two 128-token query blocks of that batch: p=c%4 and 7-p.
This balances causal attention work: every core needs exactly kv blocks
0..p (for block p) and 0..7-p (for block 7-p) = 9 useful block-units; the
kernel statically computes 12 (4 for the low block, 8 for the high block)
with data-driven 0/1 masks so the program is identical across cores (SPMD).

Per layer, each core computes q/k/v for its own 256 tokens; K^T and V are
AllGathered within each batch's 4-core group (replica_groups split), and
unpacked into absolute kv order (section j lives in core min(j,7-j), slot
0 if j<4 else 1). LayerNorm / residuals / FFN / output heads are fully
token-local. Output rows are scattered back on the host.

Matmul operands are fp16; PSUM accumulation and all vector math are fp32.
Softmax exp runs on the Act engine batched in [128,512] chunks ((N+352)
cycle cost makes small activations expensive); PSUM evacuation copies and
relu run on DVE to keep Act free for exp.
"""

import sys
import os

for _p in ("/opt/trn_rl_repo",):
    if _p not in sys.path and os.path.isdir(_p):
        sys.path.insert(0, _p)

import numpy as np

import concourse.bass as bass
import concourse.mybir as mybir
import concourse.tile as tile
from concourse import bacc
from concourse.bass_utils import run_bass_kernel_spmd
from concourse.masks import make_identity

F32 = mybir.dt.float32
AF = mybir.ActivationFunctionType
OP = mybir.AluOpType

# ---- problem constants -----------------------------------------------------
B, S, D, H, L, DFF = 2, 1024, 1024, 16, 4, 4096
DK = D // H            # 64
NOUT = 3
NC = 8                 # cores
G = 4                  # cores per batch group
T = 256                # tokens per core
TH = 2                 # 128-row tiles per core (block p, block 7-p)
DT = 8                 # D / 128
FT = DFF // 128        # 32
KB = 8                 # 128-token kv blocks per batch
NMB = 12               # static mask slots: 4 (low q-block) + 8 (high q-block)
OG = 2                 # 512-wide output column groups per 1024
LN_EPS = 1e-5

_CACHE = {}


def _build(dt_mm, no_ag=False, no_attn=False):
    nc = bacc.Bacc("TRN2", target_bir_lowering=False, debug=False,
                   enable_asserts=False, num_devices=NC)

    def din(name, shape, dt=dt_mm):
        return nc.dram_tensor(name, shape, dt, kind="ExternalInput").ap()

    # per-core inputs
    src = din("src", [128, TH], F32)
    pe = din("pe", [128, TH, D], F32)           # pe slice + emb_b, fp32
    embw = din("embw", [1, D], F32)
    masks = din("masks", [128, NMB, 128])       # 0/1 causal masks, dt_mm
    # replicated weights (dt_mm)
    Wq = din("Wq", [L, D, D])
    Wk = din("Wk", [L, D, D])
    Wv = din("Wv", [L, D, D])
    Wo = din("Wo", [L, D, D])
    fc1w = din("fc1w", [L, D, DFF])
    fc2w = din("fc2w", [L, DFF, D])
    hw1 = din("hw1", [NOUT, D, D])
    hw2 = din("hw2", [128, NOUT, DT], F32)      # hw2[o, ft*128+p, 0] -> [p, o, ft]
    out = nc.dram_tensor("y", [T, NOUT], F32, kind="ExternalOutput").ap()

    with tile.TileContext(nc) as tc:
        with (
            tc.tile_pool(name="persist", bufs=1) as pers,
            tc.tile_pool(name="xpool", bufs=2) as xpool,
            tc.tile_pool(name="hot", bufs=2) as hot,        # fp32 [128,TH,D]
            tc.tile_pool(name="ex", bufs=4) as exp_pool,
            tc.tile_pool(name="wpan", bufs=3) as wpan,      # [128, DT, 128] panels
            tc.tile_pool(name="wbig", bufs=2) as wbig,      # [128, DT, 512/1024] panels
            tc.tile_pool(name="wblk", bufs=6) as wblk,      # fc2 [128, 512] blocks
            tc.tile_pool(name="small", bufs=4) as small,
            tc.tile_pool(name="psc", bufs=2, space="PSUM") as psc,   # [128,512]
            tc.tile_pool(name="ppv", bufs=2, space="PSUM") as ppv,   # [128,256]
            tc.tile_pool(name="pmm", bufs=2, space="PSUM") as pmm,   # [128,512]
            tc.tile_pool(name="dram", bufs=1, space="DRAM") as dram,
        ):
            # ---- persistent tiles ----
            ident = pers.tile([128, 128], F32)
            make_identity(nc, ident[:])
            src_sb = pers.tile([128, TH], F32)
            nc.sync.dma_start(src_sb[:], src[:])
            embw_sb = pers.tile([1, D], F32)
            nc.sync.dma_start(embw_sb[:], embw[:])
            embw_bc = pers.tile([128, D], F32)
            nc.gpsimd.partition_broadcast(embw_bc[:], embw_sb[:])
            mask_sb = pers.tile([128, NMB, 128], dt_mm)
            nc.sync.dma_start(mask_sb[:], masks[:])
            hw2_sb = pers.tile([128, NOUT, DT], F32)
            nc.sync.dma_start(hw2_sb[:], hw2[:])

            kT_full = pers.tile([128, DT, 1024], dt_mm)     # [d%128, d//128, kv tok]
            v_ext = pers.tile([128, KB, H * 65], dt_mm)     # per head: 64 v dims + ones col
            v_ext_r = v_ext[:].rearrange("p k (h e) -> p k h e", e=65)
            nc.vector.memset(v_ext_r[:, :, :, 64:65], 1.0)

            qT = pers.tile([128, DT, T], dt_mm)
            attnT = pers.tile([128, DT, T], dt_mm)
            xT = pers.tile([128, DT, T], dt_mm)
            ff1T = pers.tile([128, FT, T], dt_mm)

            # dram scratch for collectives (4-rank groups cannot use Shared
            # outputs — bass requires >4 cores for that — so Local buffers)
            ag_k_ins = [dram.tile([D, T], dt_mm, tag=f"agki{i}", name=f"agki{i}")
                        for i in range(L)]
            ag_k_outs = [dram.tile([G * D, T], dt_mm,
                                   tag=f"agko{i}", name=f"agko{i}")
                         for i in range(L)]
            ag_v_ins = [dram.tile([T, D], dt_mm, tag=f"agvi{i}", name=f"agvi{i}")
                        for i in range(L)]
            ag_v_outs = [dram.tile([G * T, D], dt_mm,
                                   tag=f"agvo{i}", name=f"agvo{i}")
                         for i in range(L)]

            GROUPS = [[0, 1, 2, 3], [4, 5, 6, 7]]

            # ---- embedding: x = src*emb_w + (pe + emb_b) ----
            x = xpool.tile([128, TH, D], F32, tag="x")
            pe_sb = hot.tile([128, TH, D], F32, tag="hot")
            nc.sync.dma_start(pe_sb[:], pe[:])
            for th in range(TH):
                nc.vector.scalar_tensor_tensor(
                    x[:, th, :], embw_bc[:], src_sb[:, th:th + 1], pe_sb[:, th, :],
                    OP.mult, OP.add)

            def transpose_to(dst, src_x):
                # src_x fp32 [128, TH, D] -> dst dt_mm [128, DT, T] (xT layout)
                for th in range(TH):
                    for dt_i in range(DT):
                        tp = psc.tile([128, 128], F32, tag="sc")
                        nc.tensor.transpose(
                            tp[:], src_x[:, th, dt_i * 128:(dt_i + 1) * 128], ident[:])
                        nc.vector.tensor_copy(
                            dst[:, dt_i, th * 128:(th + 1) * 128], tp[:])

            def ln_inplace(y_t, resid, x_new):
                # x_new = LN(y_t) + resid   (gamma=1, beta=0)
                for th in range(TH):
                    st = small.tile([128, 2, 6], F32, tag="st")
                    nc.vector.bn_stats(st[:, 0, :], y_t[:, th, 0:512])
                    nc.vector.bn_stats(st[:, 1, :], y_t[:, th, 512:1024])
                    ag = small.tile([128, 2], F32, tag="ag")
                    nc.vector.bn_aggr(ag[:], st[:])
                    veps = small.tile([128, 1], F32, tag="veps")
                    nc.vector.tensor_scalar_add(veps[:], ag[:, 1:2], LN_EPS)
                    sd = small.tile([128, 1], F32, tag="sd")
                    nc.scalar.sqrt(sd[:], veps[:])
                    rstd = small.tile([128, 1], F32, tag="rstd")
                    nc.vector.reciprocal(rstd[:], sd[:])
                    xh = small.tile([128, D], F32, tag="xh", bufs=2)
                    nc.vector.tensor_scalar(
                        xh[:], y_t[:, th, :], ag[:, 0:1], rstd[:],
                        OP.subtract, OP.mult)
                    nc.vector.tensor_add(x_new[:, th, :], xh[:], resid[:, th, :])

            for l in range(L):
                ag_k_in, ag_k_out = ag_k_ins[l], ag_k_outs[l]
                ag_v_in, ag_v_out = ag_v_ins[l], ag_v_outs[l]
                with nc.named_scope(f"L{l}_qkv"):
                    transpose_to(xT, x)

                    # kT[dq, t] = sum_k Wk[k, dq] * xT[k, t]
                    panK = wbig.tile([128, DT, 1024], dt_mm, tag="wbig")
                    nc.sync.dma_start(
                        panK[:], Wk[l].rearrange("(kt p) m -> p kt m", p=128))
                    for dq in range(DT):
                        pmk = pmm.tile([128, 512], F32, tag="mm")
                        for kt in range(DT):
                            nc.tensor.matmul(
                                pmk[:, 0:T],
                                panK[:, kt, dq * 128:(dq + 1) * 128],
                                xT[:, kt, :],
                                start=(kt == 0), stop=(kt == DT - 1))
                        kts = small.tile([128, T], dt_mm, tag="kts", bufs=2)
                        nc.vector.tensor_copy(kts[:], pmk[:, 0:T])
                        nc.sync.dma_start(
                            ag_k_in[dq * 128:(dq + 1) * 128, :], kts[:])
                    if not no_ag:
                        nc.gpsimd.collective_compute(
                            "AllGather", OP.bypass, replica_groups=GROUPS,
                            ins=[ag_k_in.opt()], outs=[ag_k_out.opt()])
                        # unpack K^T into absolute kv order: section j of my
                        # batch lives in group-core min(j, 7-j), slot j//4
                        for j in range(KB):
                            sec = j if j < 4 else 7 - j
                            slot = 0 if j < 4 else 1
                            nc.sync.dma_start(
                                kT_full[:, :, j * 128:(j + 1) * 128],
                                ag_k_out[sec * D:(sec + 1) * D,
                                         slot * 128:(slot + 1) * 128]
                                .rearrange("(dt p) t -> p dt t", p=128))

                    # v[t, dv] = sum_k xT[k, t] * Wv[k, dv]
                    for og in range(OG):
                        pan = wbig.tile([128, DT, 512], dt_mm, tag="wbig")
                        nc.sync.dma_start(
                            pan[:],
                            Wv[l].rearrange("(kt p) n -> p kt n", p=128)
                            [:, :, og * 512:(og + 1) * 512])
                        for th in range(TH):
                            pmv = pmm.tile([128, 512], F32, tag="mm")
                            for kt in range(DT):
                                nc.tensor.matmul(
                                    pmv[:], xT[:, kt, th * 128:(th + 1) * 128],
                                    pan[:, kt, :],
                                    start=(kt == 0), stop=(kt == DT - 1))
                            vts = small.tile([128, 512], dt_mm, tag="vts", bufs=2)
                            nc.vector.tensor_copy(vts[:], pmv[:])
                            nc.sync.dma_start(
                                ag_v_in.rearrange("(a b) d -> b a d", a=TH)
                                [:, th, og * 512:(og + 1) * 512], vts[:])
                    if not no_ag:
                        nc.gpsimd.collective_compute(
                            "AllGather", OP.bypass, replica_groups=GROUPS,
                            ins=[ag_v_in.opt()], outs=[ag_v_out.opt()])
                        for j in range(KB):
                            sec = j if j < 4 else 7 - j
                            slot = 0 if j < 4 else 1
                            nc.sync.dma_start(
                                v_ext_r[:, j, :, 0:64],
                                ag_v_out[sec * T + slot * 128:
                                         sec * T + (slot + 1) * 128, :]
                                .rearrange("p (h e) -> p h e", e=64))

                    # qT
                    panQ = wbig.tile([128, DT, 1024], dt_mm, tag="wbig")
                    nc.sync.dma_start(
                        panQ[:], Wq[l].rearrange("(kt p) m -> p kt m", p=128))
                    for dq in range(DT):
                        pmq = pmm.tile([128, 512], F32, tag="mm")
                        for kt in range(DT):
                            nc.tensor.matmul(
                                pmq[:, 0:T],
                                panQ[:, kt, dq * 128:(dq + 1) * 128],
                                xT[:, kt, :],
                                start=(kt == 0), stop=(kt == DT - 1))
                        nc.vector.tensor_copy(qT[:, dq, :], pmq[:, 0:T])

                with nc.named_scope(f"L{l}_attn"):
                    # per head: q-block A (th0, kv blocks 0..3 in one
                    # [128,512] chunk) and q-block B (th1, kv blocks 0..7 in
                    # two chunks). Masks (0/1, incl. fully-off pad blocks)
                    # come from mask_sb slots 0..3 (A) and 4..11 (B).
                    if no_attn:
                        nc.vector.memset(attnT[:], 0.001)
                    for h in (range(0) if no_attn else range(H)):
                        hq, hd = (h % 2) * 64, h // 2
                        pv = ppv.tile([128, T], F32, tag="pv")
                        # (qcol, kv-chunk, mask slot, first, last)
                        chunks = [(0, 0, 0, True, True),
                                  (128, 0, 4, True, False),
                                  (128, 4, 8, False, True)]
                        for qc, kc, ms, first, last in chunks:
                            sc = psc.tile([128, 512], F32, tag="sc")
                            for kb in range(4):
                                nc.tensor.matmul(
                                    sc[:, kb * 128:(kb + 1) * 128],
                                    kT_full[hq:hq + 64, hd,
                                            (kc + kb) * 128:(kc + kb + 1) * 128],
                                    qT[hq:hq + 64, hd, qc:qc + 128],
                                    start=True, stop=True)
                            ex = exp_pool.tile([128, 512], dt_mm, tag="ex")
                            nc.scalar.activation(ex[:], sc[:], AF.Exp, scale=0.125)
                            nc.vector.tensor_mul(
                                ex[:], ex[:],
                                mask_sb[:, ms:ms + 4, :]
                                .rearrange("p a b -> p (a b)"))
                            for kb in range(4):
                                nc.tensor.matmul(
                                    pv[0:65, qc:qc + 128],
                                    v_ext_r[:, kc + kb, h, :],
                                    ex[:, kb * 128:(kb + 1) * 128],
                                    start=(first and kb == 0),
                                    stop=(last and kb == 3),
                                    skip_group_check=True)
                        den = small.tile([1, T], F32, tag="den")
                        nc.vector.tensor_scalar_add(den[:], pv[64:65, :], 1e-9)
                        rcp = small.tile([1, T], F32, tag="rcp")
                        nc.vector.reciprocal(rcp[:], den[:])
                        rb = small.tile([128, T], F32, tag="rb", bufs=2)
                        nc.gpsimd.partition_broadcast(rb[:], rcp[:])
                        nc.vector.tensor_tensor(
                            attnT[hq:hq + 64, hd, :], pv[0:64, :],
                            rb[hq:hq + 64, :], OP.mult)

                with nc.named_scope(f"L{l}_wo_ln1"):
                    attnfull = hot.tile([128, TH, D], F32, tag="hot")
                    for og in range(OG):
                        pan = wbig.tile([128, DT, 512], dt_mm, tag="wbig")
                        nc.sync.dma_start(
                            pan[:],
                            Wo[l].rearrange("(kt p) n -> p kt n", p=128)
                            [:, :, og * 512:(og + 1) * 512])
                        for th in range(TH):
                            pmo = pmm.tile([128, 512], F32, tag="mm")
                            for kt in range(DT):
                                nc.tensor.matmul(
                                    pmo[:], attnT[:, kt, th * 128:(th + 1) * 128],
                                    pan[:, kt, :],
                                    start=(kt == 0), stop=(kt == DT - 1))
                            nc.vector.tensor_copy(
                                attnfull[:, th, og * 512:(og + 1) * 512], pmo[:])
                    y_t = hot.tile([128, TH, D], F32, tag="hot2")
                    for th in range(TH):
                        nc.vector.tensor_add(
                            y_t[:, th, :], x[:, th, :], attnfull[:, th, :])
                    x = xpool.tile([128, TH, D], F32, tag="x")
                    ln_inplace(y_t, attnfull, x)

                with nc.named_scope(f"L{l}_ffn"):
                    transpose_to(xT, x)
                    # ff1T[f, t] = relu(sum_k fc1w[k, f] * xT[k, t])
                    for ft in range(FT):
                        pan = wpan.tile([128, DT, 128], dt_mm, tag="wpan")
                        nc.sync.dma_start(
                            pan[:],
                            fc1w[l].rearrange("(kt p) m -> p kt m", p=128)
                            [:, :, ft * 128:(ft + 1) * 128])
                        pmf = pmm.tile([128, 512], F32, tag="mm")
                        for kt in range(DT):
                            nc.tensor.matmul(
                                pmf[:, 0:T], pan[:, kt, :], xT[:, kt, :],
                                start=(kt == 0), stop=(kt == DT - 1))
                        nc.vector.tensor_scalar_max(
                            ff1T[:, ft, :], pmf[:, 0:T], 0.0)
                    # ff = relu(ff1 @ fc2w); fc2 blocks fetched once per
                    # (og, dft), both th matmuls share the fetch
                    ff = hot.tile([128, TH, D], F32, tag="hot")
                    for og in range(OG):
                        pmf2 = [pmm.tile([128, 512], F32, tag="mm",
                                         name=f"pmf2_{l}_{og}_{th_i}")
                                for th_i in range(TH)]
                        for dft in range(FT):
                            blk = wblk.tile([128, 512], dt_mm, tag="wblk")
                            nc.sync.dma_start(
                                blk[:],
                                fc2w[l, dft * 128:(dft + 1) * 128,
                                     og * 512:(og + 1) * 512])
                            for th in range(TH):
                                nc.tensor.matmul(
                                    pmf2[th][:],
                                    ff1T[:, dft, th * 128:(th + 1) * 128],
                                    blk[:],
                                    start=(dft == 0), stop=(dft == FT - 1))
                        for th in range(TH):
                            nc.vector.tensor_scalar_max(
                                ff[:, th, og * 512:(og + 1) * 512],
                                pmf2[th][:], 0.0)
                    y2 = hot.tile([128, TH, D], F32, tag="hot2")
                    for th in range(TH):
                        nc.vector.tensor_add(
                            y2[:, th, :], x[:, th, :], ff[:, th, :])
                    x = xpool.tile([128, TH, D], F32, tag="x")
                    ln_inplace(y2, ff, x)

            # ---- output heads ----
            with nc.named_scope("heads"):
                transpose_to(xT, x)
                out_sb = pers.tile([128, TH, NOUT], F32)
                for o in range(NOUT):
                    for ft in range(DT):
                        pan = wpan.tile([128, DT, 128], dt_mm, tag="wpan")
                        nc.sync.dma_start(
                            pan[:],
                            hw1[o].rearrange("(kt p) m -> p kt m", p=128)
                            [:, :, ft * 128:(ft + 1) * 128])
                        pmh = pmm.tile([128, 512], F32, tag="mm")
                        for kt in range(DT):
                            nc.tensor.matmul(
                                pmh[:, 0:T], pan[:, kt, :], xT[:, kt, :],
                                start=(kt == 0), stop=(kt == DT - 1))
                        nc.vector.tensor_scalar_max(
                            ff1T[:, ft, :], pmh[:, 0:T], 0.0)
                    # hw2 rhs is fp32; cast to dt_mm for the matmul
                    w2 = small.tile([128, DT], dt_mm, tag="w2")
                    nc.vector.tensor_copy(w2[:], hw2_sb[:, o, :])
                    for th in range(TH):
                        pho = psc.tile([128, 128], F32, tag="sc")
                        for ft in range(DT):
                            nc.tensor.matmul(
                                pho[:, 0:1], ff1T[:, ft, th * 128:(th + 1) * 128],
                                w2[:, ft:ft + 1],
                                start=(ft == 0), stop=(ft == DT - 1))
                        nc.vector.tensor_copy(out_sb[:, th, o:o + 1], pho[:, 0:1])
                nc.sync.dma_start(
                    out[:].rearrange("(a b) o -> b a o", a=TH), out_sb[:])

    nc.compile()
    return nc


def _prep_inputs(inputs, dt_np):
    """Build the 8 per-core input maps from the full-problem inputs."""
    as_np = {k: np.asarray(v) for k, v in inputs.items()}
    g = as_np

    # specialization guard: biases / LN affine params are identity in this
    # problem (spec fills); the device program omits them.
    for name in ("bq", "bk", "bv", "bo", "fc1_b", "fc2_b", "hb1", "hb2",
                 "emb_b", "ln1_b", "ln2_b"):
        assert not np.any(g[name]), f"{name} must be zero for this kernel"
    for name in ("ln1_g", "ln2_g"):
        assert np.all(g[name] == 1.0), f"{name} must be ones for this kernel"

    wq = g["Wq"].astype(dt_np)
    wk = g["Wk"].astype(dt_np)
    wv = g["Wv"].astype(dt_np)
    wo = g["Wo"].astype(dt_np)
    fc1 = g["fc1_w"].astype(dt_np)
    fc2 = g["fc2_w"].astype(dt_np)
    hw1 = g["hw1"].astype(dt_np)
    hw2 = np.transpose(g["hw2"][:, :, 0].reshape(NOUT, DT, 128), (2, 0, 1))
    hw2 = np.ascontiguousarray(hw2, dtype=np.float32)
    embw = g["emb_w"].astype(np.float32)
    pe_full = g["pe"].astype(np.float32) + g["emb_b"][None, :].astype(np.float32)

    tril = (np.arange(128)[:, None] <= np.arange(128)[None, :])

    in_maps = []
    for c in range(NC):
        b, p = c // 4, c % 4
        blkA, blkB = p, 7 - p
        rows = np.r_[blkA * 128:(blkA + 1) * 128, blkB * 128:(blkB + 1) * 128]
        src_c = g["src"][b, rows, 0].astype(np.float32)        # [256]
        src_sb = np.ascontiguousarray(src_c.reshape(TH, 128).T)  # [128, TH]
        pe_c = pe_full[rows]                                    # [256, 1024]
        pe_sb = np.ascontiguousarray(
            np.transpose(pe_c.reshape(TH, 128, D), (1, 0, 2)))
        # masks [128 kv_p, NMB, 128 q]: slots 0..3 -> q-block A (diag at p),
        # slots 4..11 -> q-block B (diag at 7-p)
        m = np.zeros((128, NMB, 128), dtype=dt_np)
        for s in range(4):
            if s < blkA:
                m[:, s, :] = 1.0
            elif s == blkA:
                m[:, s, :] = tril
        for s in range(8):
            if s < blkB:
                m[:, 4 + s, :] = 1.0
            elif s == blkB:
                m[:, 4 + s, :] = tril
        in_maps.append({
            "src": src_sb, "pe": pe_sb, "embw": embw,
            "masks": np.ascontiguousarray(m),
            "Wq": wq, "Wk": wk, "Wv": wv, "Wo": wo,
            "fc1w": fc1, "fc2w": fc2, "hw1": hw1, "hw2": hw2,
        })
    return in_maps


def _make_runner(nc):
    """Build the 8-core jitted PJRT callable once (same lowering path as
    run_bass_kernel_spmd under axon, but reusable across calls)."""
    import jax
    from jax.sharding import Mesh, PartitionSpec, NamedSharding
    from jax.experimental.shard_map import shard_map
    from concourse import bass2jax

    bass2jax.install_neuronx_cc_hook()
    partition_name = (nc.partition_id_tensor.name
                      if nc.partition_id_tensor else None)
    in_names, out_names, out_avals, zero_outs = [], [], [], []
    for alloc in nc.m.functions[0].allocations:
        if not isinstance(alloc, mybir.MemoryLocationSet):
            continue
        name = alloc.memorylocations[0].name
        if alloc.kind == "ExternalInput":
            if name != partition_name:
                in_names.append(name)
        elif alloc.kind == "ExternalOutput":
            out_names.append(name)
            shape = tuple(alloc.tensor_shape)
            dtype = mybir.dt.np(alloc.dtype)
            out_avals.append(jax.core.ShapedArray(shape, dtype))
            zero_outs.append(np.zeros(shape, dtype))
    all_in_names = list(in_names) + list(out_names)
    if partition_name is not None:
        all_in_names.append(partition_name)

    def _body(*args):
        operands = list(args)
        if partition_name is not None:
            operands.append(bass2jax.partition_id_tensor())
        outs = bass2jax._bass_exec_p.bind(
            *operands, out_avals=tuple(out_avals),
            in_names=tuple(all_in_names), out_names=tuple(out_names),
            lowering_input_output_aliases=(), sim_require_finite=True,
            sim_require_nnan=True, nc=nc)
        return tuple(outs)

    devices = jax.devices()[:NC]
    mesh = Mesh(np.asarray(devices), ("core",))
    n_args = len(in_names) + len(out_names)
    fn = jax.jit(shard_map(_body, mesh=mesh,
                           in_specs=(PartitionSpec("core"),) * n_args,
                           out_specs=(PartitionSpec("core"),) * len(out_names),
                           check_rep=False),
                 keep_unused=True)
    sharding = NamedSharding(mesh, PartitionSpec("core"))
    return fn, in_names, out_names, zero_outs, sharding


def _run_fast(nc, in_maps):
    """Execute with device-resident cached inputs; returns [T, NOUT] per core."""
    import jax
    import hashlib

    if "runner" not in _CACHE:
        _CACHE["runner"] = _make_runner(nc)
    fn, in_names, out_names, zero_outs, sharding = _CACHE["runner"]

    h = hashlib.sha1()
    for name in in_names:
        for c in range(NC):
            h.update(np.ascontiguousarray(in_maps[c][name]).tobytes())
    digest = h.hexdigest()
    if _CACHE.get("args_key") != digest:
        concat_in = [np.concatenate([np.asarray(in_maps[c][i])
                                     for c in range(NC)], axis=0)
                     for i in in_names]
        concat_zeros = [np.zeros((NC * z.shape[0], *z.shape[1:]), z.dtype)
                        for z in zero_outs]
        args = [jax.device_put(a, sharding) for a in concat_in + concat_zeros]
        jax.block_until_ready(args)
        _CACHE["args"] = args
        _CACHE["args_key"] = digest
    outs = fn(*_CACHE["args"])
    y = np.asarray(outs[out_names.index("y")])
    return y.reshape(NC, T, NOUT)


def kernel(**inputs) -> np.ndarray:
    dt_mm = mybir.dt.float16
    dt_np = np.float16
    key = ("prog", str(dt_mm))
    if key not in _CACHE:
        _CACHE[key] = _build(dt_mm)
    nc = _CACHE[key]
    in_maps = _prep_inputs(inputs, dt_np)
    try:
        per_core = _run_fast(nc, in_maps)
    except Exception:
        res = run_bass_kernel_spmd(nc, in_maps, core_ids=list(range(NC)))
        per_core = np.stack([res.results[c]["y"] for c in range(NC)])
    full = np.zeros((B, S, NOUT), dtype=np.float32)
    for c in range(NC):
        b, p = c // 4, c % 4
        blkA, blkB = p, 7 - p
        full[b, blkA * 128:(blkA + 1) * 128, :] = per_core[c][0:128]
        full[b, blkB * 128:(blkB + 1) * 128, :] = per_core[c][128:256]
    return full


if __name__ == "__main__":
    sys.path.insert(0, os.path.dirname(os.path.abspath(__file__)))
    import reference
    ins = reference.setup_inputs()
    want = np.asarray(reference.reference(**ins))
    got = kernel(**{k: np.asarray(v) for k, v in ins.items()})
    err = np.abs(got - want).max() / np.abs(want).max()
    print("Relative error:", err)


# revision 15
# speedup vs baseline: 1.0336x; 1.0336x over previous
"""Bass/Tile TRN2 kernel for nn_Decoder_Transformer (B=2, S=1024, D=1024, H=16,
L=4, DFF=4096, 3 output heads) on 8 NeuronCores.

Sharding: balanced causal sequence-parallel ("zebra"). Core c serves batch
b=c//4 and owns two 128-token query blocks of that batch: p=c%4 and 7-p.
This balances causal attention work: every core needs exactly kv blocks
0..p (for block p) and 0..7-p (for block 7-p) = 9 useful block-units; the
kernel statically computes 12 (4 for the low block, 8 for the high block)
with data-driven 0/1 masks so the program is identical across cores (SPMD).

Per layer, each core computes q/k/v for its own 256 tokens; K^T and V are
AllGathered within each batch's 4-core group (replica_groups split), and
unpacked into absolute kv order (section j lives in core min(j,7-j), slot
0 if j<4 else 1). LayerNorm / residuals / FFN / output heads are fully
token-local. Output rows are scattered back on the host.

Matmul operands are fp16; PSUM accumulation and all vector math are fp32.
Softmax exp runs on the Act engine batched in [128,512] chunks ((N+352)
cycle cost makes small activations expensive); PSUM evacuation copies and
relu run on DVE to keep Act free for exp.
"""

import sys
import os

for _p in ("/opt/trn_rl_repo",):
    if _p not in sys.path and os.path.isdir(_p):
        sys.path.insert(0, _p)

import numpy as np

import concourse.bass as bass
import concourse.mybir as mybir
import concourse.tile as tile
from concourse import bacc
from concourse.bass_utils import run_bass_kernel_spmd
from concourse.masks import make_identity

F32 = mybir.dt.float32
AF = mybir.ActivationFunctionType
OP = mybir.AluOpType

# ---- problem constants -----------------------------------------------------
B, S, D, H, L, DFF = 2, 1024, 1024, 16, 4, 4096
DK = D // H            # 64
NOUT = 3
NC = 8                 # cores
G = 4                  # cores per batch group
T = 256                # tokens per core
TH = 2                 # 128-row tiles per core (block p, block 7-p)
DT = 8                 # D / 128
FT = DFF // 128        # 32
KB = 8                 # 128-token kv blocks per batch
NMB = 24               # mask slots (chunk1 x2 sub-pairs x2 heads + chunk2 x2)
OG = 2                 # 512-wide output column groups per 1024
LN_EPS = 1e-5

_CACHE = {}


def _build(dt_mm, no_ag=False, no_attn=False):
    nc = bacc.Bacc("TRN2", target_bir_lowering=False, debug=False,
                   enable_asserts=False, num_devices=NC)

    def din(name, shape, dt=dt_mm):
        return nc.dram_tensor(name, shape, dt, kind="ExternalInput").ap()

    # per-core inputs
    src = din("src", [128, TH], F32)
    pe = din("pe", [128, TH, D], F32)           # pe slice + emb_b, fp32
    embw = din("embw", [1, D], F32)
    masks = din("masks", [128, NMB, 128])       # 0/1 causal masks, dt_mm
    # replicated weights (dt_mm)
    Wq = din("Wq", [L, D, D])
    Wk = din("Wk", [L, D, D])
    Wv = din("Wv", [L, D, D])
    Wo = din("Wo", [L, D, D])
    fc1w = din("fc1w", [L, D, DFF])
    fc2w = din("fc2w", [L, DFF, D])
    hw1 = din("hw1", [NOUT, D, D])
    hw2 = din("hw2", [128, NOUT, DT], F32)      # hw2[o, ft*128+p, 0] -> [p, o, ft]
    out = nc.dram_tensor("y", [T, NOUT], F32, kind="ExternalOutput").ap()

    with tile.TileContext(nc) as tc:
        with (
            tc.tile_pool(name="persist", bufs=1) as pers,
            tc.tile_pool(name="xpool", bufs=2) as xpool,
            tc.tile_pool(name="hot", bufs=2) as hot,        # fp32 [128,TH,D]
            tc.tile_pool(name="ex", bufs=4) as exp_pool,
            tc.tile_pool(name="wpan", bufs=3) as wpan,      # [128, DT, 128] panels
            tc.tile_pool(name="wbig", bufs=2) as wbig,      # [128, DT, 512/1024] panels
            tc.tile_pool(name="wblk", bufs=6) as wblk,      # fc2 [128, 512] blocks
            tc.tile_pool(name="small", bufs=4) as small,
            tc.tile_pool(name="psc", bufs=2, space="PSUM") as psc,   # [128,1024]
            tc.tile_pool(name="ppv", bufs=2, space="PSUM") as ppv,   # [128,256]
            tc.tile_pool(name="pmm", bufs=2, space="PSUM") as pmm,   # [128,512]
            tc.tile_pool(name="dram", bufs=1, space="DRAM") as dram,
        ):
            # ---- persistent tiles ----
            ident = pers.tile([128, 128], F32)
            make_identity(nc, ident[:])
            src_sb = pers.tile([128, TH], F32)
            nc.sync.dma_start(src_sb[:], src[:])
            embw_sb = pers.tile([1, D], F32)
            nc.sync.dma_start(embw_sb[:], embw[:])
            embw_bc = pers.tile([128, D], F32)
            nc.gpsimd.partition_broadcast(embw_bc[:], embw_sb[:])
            mask_sb = pers.tile([128, NMB, 128], dt_mm)
            nc.sync.dma_start(mask_sb[:], masks[:])
            hw2_sb = pers.tile([128, NOUT, DT], F32)
            nc.sync.dma_start(hw2_sb[:], hw2[:])

            kT_full = pers.tile([128, DT, 1024], dt_mm)     # [d%128, d//128, kv tok]
            v_ext = pers.tile([128, KB, H * 65], dt_mm)     # per head: 64 v dims + ones col
            v_ext_r = v_ext[:].rearrange("p k (h e) -> p k h e", e=65)
            nc.vector.memset(v_ext_r[:, :, :, 64:65], 1.0)
            if no_ag:  # ablation: no collectives -> fill kv locally
                nc.vector.memset(kT_full[:], 0.001)
                nc.vector.memset(v_ext_r[:, :, :, 0:64], 0.001)

            qT = pers.tile([128, DT, T], dt_mm)
            attnT = pers.tile([128, DT, T], dt_mm)
            xT = pers.tile([128, DT, T], dt_mm)
            ff1T = pers.tile([128, FT, T], dt_mm)

            # dram scratch for collectives (4-rank groups cannot use Shared
            # outputs — bass requires >4 cores for that — so Local buffers).
            # One combined K+V AllGather per layer: rows 0:1024 = K^T
            # [dq, tok]; rows 1024:2048 = V packed col-group-major:
            # row 1024 + g*256 + tok, col c  <->  V[tok, g*256+c].
            ag_ins = [dram.tile([2 * D, T], dt_mm, tag=f"agi{i}", name=f"agi{i}")
                      for i in range(L)]
            ag_outs = [dram.tile([G * 2 * D, T], dt_mm,
                                 tag=f"ago{i}", name=f"ago{i}")
                       for i in range(L)]

            GROUPS = [[0, 1, 2, 3], [4, 5, 6, 7]]

            # ---- embedding: x = src*emb_w + (pe + emb_b) ----
            x = xpool.tile([128, TH, D], F32, tag="x")
            pe_sb = hot.tile([128, TH, D], F32, tag="hot")
            nc.sync.dma_start(pe_sb[:], pe[:])
            for th in range(TH):
                nc.vector.scalar_tensor_tensor(
                    x[:, th, :], embw_bc[:], src_sb[:, th:th + 1], pe_sb[:, th, :],
                    OP.mult, OP.add)

            def transpose_to(dst, src_x):
                # src_x fp32 [128, TH, D] -> dst dt_mm [128, DT, T] (xT layout)
                for th in range(TH):
                    for dt_i in range(DT):
                        tp = psc.tile([128, 128], F32, tag="sc")
                        nc.tensor.transpose(
                            tp[:], src_x[:, th, dt_i * 128:(dt_i + 1) * 128], ident[:])
                        nc.vector.tensor_copy(
                            dst[:, dt_i, th * 128:(th + 1) * 128], tp[:])

            def ln_inplace(y_t, resid, x_new):
                # x_new = LN(y_t) + resid   (gamma=1, beta=0)
                for th in range(TH):
                    st = small.tile([128, 2, 6], F32, tag="st")
                    nc.vector.bn_stats(st[:, 0, :], y_t[:, th, 0:512])
                    nc.vector.bn_stats(st[:, 1, :], y_t[:, th, 512:1024])
                    ag = small.tile([128, 2], F32, tag="ag")
                    nc.vector.bn_aggr(ag[:], st[:])
                    veps = small.tile([128, 1], F32, tag="veps")
                    nc.vector.tensor_scalar_add(veps[:], ag[:, 1:2], LN_EPS)
                    sd = small.tile([128, 1], F32, tag="sd")
                    nc.scalar.sqrt(sd[:], veps[:])
                    rstd = small.tile([128, 1], F32, tag="rstd")
                    nc.vector.reciprocal(rstd[:], sd[:])
                    xh = small.tile([128, D], F32, tag="xh", bufs=2)
                    nc.vector.tensor_scalar(
                        xh[:], y_t[:, th, :], ag[:, 0:1], rstd[:],
                        OP.subtract, OP.mult)
                    nc.vector.tensor_add(x_new[:, th, :], xh[:], resid[:, th, :])

            for l in range(L):
                ag_in, ag_out = ag_ins[l], ag_outs[l]
                with nc.named_scope(f"L{l}_qkv"):
                    transpose_to(xT, x)

                    # kT[dq, t] = sum_k Wk[k, dq] * xT[k, t]
                    panK = wbig.tile([128, DT, 1024], dt_mm, tag="wbig")
                    nc.sync.dma_start(
                        panK[:], Wk[l].rearrange("(kt p) m -> p kt m", p=128))
                    for dq in range(DT):
                        pmk = pmm.tile([128, 512], F32, tag="mm")
                        for kt in range(DT):
                            nc.tensor.matmul(
                                pmk[:, 0:T],
                                panK[:, kt, dq * 128:(dq + 1) * 128],
                                xT[:, kt, :],
                                start=(kt == 0), stop=(kt == DT - 1))
                        kts = small.tile([128, T], dt_mm, tag="kts", bufs=2)
                        nc.vector.tensor_copy(kts[:], pmk[:, 0:T])
                        nc.sync.dma_start(
                            ag_in[dq * 128:(dq + 1) * 128, :], kts[:])

                    # v[t, dv] = sum_k xT[k, t] * Wv[k, dv]
                    for og in range(OG):
                        pan = wbig.tile([128, DT, 512], dt_mm, tag="wbig")
                        nc.sync.dma_start(
                            pan[:],
                            Wv[l].rearrange("(kt p) n -> p kt n", p=128)
                            [:, :, og * 512:(og + 1) * 512])
                        for th in range(TH):
                            pmv = pmm.tile([128, 512], F32, tag="mm")
                            for kt in range(DT):
                                nc.tensor.matmul(
                                    pmv[:], xT[:, kt, th * 128:(th + 1) * 128],
                                    pan[:, kt, :],
                                    start=(kt == 0), stop=(kt == DT - 1))
                            vts = small.tile([128, 512], dt_mm, tag="vts", bufs=2)
                            nc.vector.tensor_copy(vts[:], pmv[:])
                            # vts cols [0:256]/[256:512] -> col-groups
                            # g = 2*og / 2*og+1 of the V region
                            for gg in range(2):
                                nc.sync.dma_start(
                                    ag_in[D + (2 * og + gg) * 256 + th * 128:
                                          D + (2 * og + gg) * 256 + (th + 1) * 128,
                                          :],
                                    vts[:, gg * 256:(gg + 1) * 256])
                    if not no_ag:
                        nc.gpsimd.collective_compute(
                            "AllGather", OP.bypass, replica_groups=GROUPS,
                            ins=[ag_in.opt()], outs=[ag_out.opt()])

                    # qT (overlaps the AllGather)
                    panQ = wbig.tile([128, DT, 1024], dt_mm, tag="wbig")
                    nc.sync.dma_start(
                        panQ[:], Wq[l].rearrange("(kt p) m -> p kt m", p=128))
                    for dq in range(DT):
                        pmq = pmm.tile([128, 512], F32, tag="mm")
                        for kt in range(DT):
                            nc.tensor.matmul(
                                pmq[:, 0:T],
                                panQ[:, kt, dq * 128:(dq + 1) * 128],
                                xT[:, kt, :],
                                start=(kt == 0), stop=(kt == DT - 1))
                        nc.vector.tensor_copy(qT[:, dq, :], pmq[:, 0:T])

                    if not no_ag:
                        # unpack into absolute kv order: section j of my
                        # batch lives in group-core min(j, 7-j), slot j//4
                        for j in range(KB):
                            sec = j if j < 4 else 7 - j
                            slot = 0 if j < 4 else 1
                            base = sec * 2 * D
                            nc.sync.dma_start(
                                kT_full[:, :, j * 128:(j + 1) * 128],
                                ag_out[base:base + D,
                                       slot * 128:(slot + 1) * 128]
                                .rearrange("(dt p) t -> p dt t", p=128))
                            for gg in range(4):
                                rb0 = base + D + gg * 256 + slot * 128
                                nc.sync.dma_start(
                                    v_ext_r[:, j, 4 * gg:4 * gg + 4, 0:64],
                                    ag_out[rb0:rb0 + 128, :]
                                    .rearrange("p (h e) -> p h e", e=64))

                with nc.named_scope(f"L{l}_attn"):
                    # head-pair processing: heads 2pd (partitions 0:64) and
                    # 2pd+1 (64:128) share hd=pd; their score matmuls use
                    # disjoint PE row-groups (base_partition 0 vs 64) and run
                    # concurrently. kv blocks 0..3 are needed by both query
                    # blocks -> N=256 matmuls; kv 4..7 only by q-block B
                    # (cols 128:256) -> N=128. Masks (0/1, incl. fully-off
                    # pad blocks) come from mask_sb: slots 0..7 = chunk1
                    # ([A|B] per kv block), slots 8..11 = chunk2 (B only).
                    if no_attn:
                        nc.vector.memset(attnT[:], 0.001)
                    for pd in (range(0) if no_attn else range(H // 2)):
                        pvs = [ppv.tile([128, T], F32, tag="pv",
                                        name=f"pv_{l}_{pd}_{i}")
                               for i in range(2)]
                        # chunk1: kv sub-pairs (0,1) and (2,3), q = 0:256;
                        # both heads' scores in one [128,1024] PSUM tile
                        # (even head -> cols 0:512, odd -> 512:1024) so one
                        # Act exp covers the pair.
                        for sp in range(2):
                            sc = psc.tile([128, 1024], F32, tag="sc")
                            for i2 in range(2):
                                kb = 2 * sp + i2
                                for ho in range(2):
                                    nc.tensor.matmul(
                                        sc[:, ho * 512 + i2 * 256:
                                           ho * 512 + (i2 + 1) * 256],
                                        kT_full[ho * 64:ho * 64 + 64, pd,
                                                kb * 128:(kb + 1) * 128],
                                        qT[ho * 64:ho * 64 + 64, pd, :],
                                        start=True, stop=True)
                            ex = exp_pool.tile([128, 1024], dt_mm, tag="ex")
                            nc.scalar.activation(
                                ex[:], sc[:], AF.Exp, scale=0.125)
                            nc.vector.tensor_mul(
                                ex[:], ex[:],
                                mask_sb[:, 8 * sp:8 * sp + 8, :]
                                .rearrange("p a b -> p (a b)"))
                            for ho in range(2):
                                for i2 in range(2):
                                    kb = 2 * sp + i2
                                    nc.tensor.matmul(
                                        pvs[ho][0:65, :],
                                        v_ext_r[:, kb, 2 * pd + ho, :],
                                        ex[:, ho * 512 + i2 * 256:
                                           ho * 512 + (i2 + 1) * 256],
                                        start=(kb == 0), stop=(kb == 3),
                                        skip_group_check=True)
                        # chunk2: kv blocks 4..7, q-block B only (cols
                        # 128:256); continues pv accumulation on those cols
                        sc = psc.tile([128, 1024], F32, tag="sc")
                        for j in range(4):
                            for ho in range(2):
                                nc.tensor.matmul(
                                    sc[:, ho * 512 + j * 128:
                                       ho * 512 + (j + 1) * 128],
                                    kT_full[ho * 64:ho * 64 + 64, pd,
                                            (4 + j) * 128:(5 + j) * 128],
                                    qT[ho * 64:ho * 64 + 64, pd, 128:256],
                                    start=True, stop=True)
                        ex = exp_pool.tile([128, 1024], dt_mm, tag="ex")
                        nc.scalar.activation(ex[:], sc[:], AF.Exp, scale=0.125)
                        nc.vector.tensor_mul(
                            ex[:], ex[:],
                            mask_sb[:, 16:24, :]
                            .rearrange("p a b -> p (a b)"))
                        for ho in range(2):
                            for j in range(4):
                                nc.tensor.matmul(
                                    pvs[ho][0:65, 128:256],
                                    v_ext_r[:, 4 + j, 2 * pd + ho, :],
                                    ex[:, ho * 512 + j * 128:
                                       ho * 512 + (j + 1) * 128],
                                    start=False, stop=(j == 3),
                                    skip_group_check=True)
                        for ho in range(2):
                            den = small.tile([1, T], F32, tag="den")
                            nc.vector.tensor_scalar_add(
                                den[:], pvs[ho][64:65, :], 1e-9)
                            rcp = small.tile([1, T], F32, tag="rcp")
                            nc.vector.reciprocal(rcp[:], den[:])
                            rb = small.tile([128, T], F32, tag="rb", bufs=2)
                            nc.gpsimd.partition_broadcast(rb[:], rcp[:])
                            nc.vector.tensor_tensor(
                                attnT[ho * 64:ho * 64 + 64, pd, :],
                                pvs[ho][0:64, :],
                                rb[ho * 64:ho * 64 + 64, :], OP.mult)

                with nc.named_scope(f"L{l}_wo_ln1"):
                    attnfull = hot.tile([128, TH, D], F32, tag="hot")
                    for og in range(OG):
                        pan = wbig.tile([128, DT, 512], dt_mm, tag="wbig")
                        nc.sync.dma_start(
                            pan[:],
                            Wo[l].rearrange("(kt p) n -> p kt n", p=128)
                            [:, :, og * 512:(og + 1) * 512])
                        for th in range(TH):
                            pmo = pmm.tile([128, 512], F32, tag="mm")
                            for kt in range(DT):
                                nc.tensor.matmul(
                                    pmo[:], attnT[:, kt, th * 128:(th + 1) * 128],
                                    pan[:, kt, :],
                                    start=(kt == 0), stop=(kt == DT - 1))
                            nc.vector.tensor_copy(
                                attnfull[:, th, og * 512:(og + 1) * 512], pmo[:])
                    y_t = hot.tile([128, TH, D], F32, tag="hot2")
                    for th in range(TH):
                        nc.vector.tensor_add(
                            y_t[:, th, :], x[:, th, :], attnfull[:, th, :])
                    x = xpool.tile([128, TH, D], F32, tag="x")
                    ln_inplace(y_t, attnfull, x)

                with nc.named_scope(f"L{l}_ffn"):
                    transpose_to(xT, x)
                    # ff1T[f, t] = relu(sum_k fc1w[k, f] * xT[k, t])
                    for ft in range(FT):
                        pan = wpan.tile([128, DT, 128], dt_mm, tag="wpan")
                        nc.sync.dma_start(
                            pan[:],
                            fc1w[l].rearrange("(kt p) m -> p kt m", p=128)
                            [:, :, ft * 128:(ft + 1) * 128])
                        pmf = pmm.tile([128, 512], F32, tag="mm")
                        for kt in range(DT):
                            nc.tensor.matmul(
                                pmf[:, 0:T], pan[:, kt, :], xT[:, kt, :],
                                start=(kt == 0), stop=(kt == DT - 1))
                        nc.vector.tensor_scalar_max(
                            ff1T[:, ft, :], pmf[:, 0:T], 0.0)
                    # ff = relu(ff1 @ fc2w); fc2 blocks fetched once per
                    # (og, dft), both th matmuls share the fetch
                    ff = hot.tile([128, TH, D], F32, tag="hot")
                    for og in range(OG):
                        pmf2 = [pmm.tile([128, 512], F32, tag="mm",
                                         name=f"pmf2_{l}_{og}_{th_i}")
                                for th_i in range(TH)]
                        for dft in range(FT):
                            blk = wblk.tile([128, 512], dt_mm, tag="wblk")
                            nc.sync.dma_start(
                                blk[:],
                                fc2w[l, dft * 128:(dft + 1) * 128,
                                     og * 512:(og + 1) * 512])
                            for th in range(TH):
                                nc.tensor.matmul(
                                    pmf2[th][:],
                                    ff1T[:, dft, th * 128:(th + 1) * 128],
                                    blk[:],
                                    start=(dft == 0), stop=(dft == FT - 1))
                        for th in range(TH):
                            nc.vector.tensor_scalar_max(
                                ff[:, th, og * 512:(og + 1) * 512],
                                pmf2[th][:], 0.0)
                    y2 = hot.tile([128, TH, D], F32, tag="hot2")
                    for th in range(TH):
                        nc.vector.tensor_add(
                            y2[:, th, :], x[:, th, :], ff[:, th, :])
                    x = xpool.tile([128, TH, D], F32, tag="x")
                    ln_inplace(y2, ff, x)

            # ---- output heads ----
            with nc.named_scope("heads"):
                transpose_to(xT, x)
                out_sb = pers.tile([128, TH, NOUT], F32)
                for o in range(NOUT):
                    for ft in range(DT):
                        pan = wpan.tile([128, DT, 128], dt_mm, tag="wpan")
                        nc.sync.dma_start(
                            pan[:],
                            hw1[o].rearrange("(kt p) m -> p kt m", p=128)
                            [:, :, ft * 128:(ft + 1) * 128])
                        pmh = pmm.tile([128, 512], F32, tag="mm")
                        for kt in range(DT):
                            nc.tensor.matmul(
                                pmh[:, 0:T], pan[:, kt, :], xT[:, kt, :],
                                start=(kt == 0), stop=(kt == DT - 1))
                        nc.vector.tensor_scalar_max(
                            ff1T[:, ft, :], pmh[:, 0:T], 0.0)
                    # hw2 rhs is fp32; cast to dt_mm for the matmul
                    w2 = small.tile([128, DT], dt_mm, tag="w2")
                    nc.vector.tensor_copy(w2[:], hw2_sb[:, o, :])
                    for th in range(TH):
                        pho = psc.tile([128, 128], F32, tag="sc")
                        for ft in range(DT):
                            nc.tensor.matmul(
                                pho[:, 0:1], ff1T[:, ft, th * 128:(th + 1) * 128],
                                w2[:, ft:ft + 1],
                                start=(ft == 0), stop=(ft == DT - 1))
                        nc.vector.tensor_copy(out_sb[:, th, o:o + 1], pho[:, 0:1])
                nc.sync.dma_start(
                    out[:].rearrange("(a b) o -> b a o", a=TH), out_sb[:])

    nc.compile()
    return nc


def _prep_inputs(inputs, dt_np):
    """Build the 8 per-core input maps from the full-problem inputs."""
    as_np = {k: np.asarray(v) for k, v in inputs.items()}
    g = as_np

    # specialization guard: biases / LN affine params are identity in this
    # problem (spec fills); the device program omits them.
    for name in ("bq", "bk", "bv", "bo", "fc1_b", "fc2_b", "hb1", "hb2",
                 "emb_b", "ln1_b", "ln2_b"):
        assert not np.any(g[name]), f"{name} must be zero for this kernel"
    for name in ("ln1_g", "ln2_g"):
        assert np.all(g[name] == 1.0), f"{name} must be ones for this kernel"

    wq = g["Wq"].astype(dt_np)
    wk = g["Wk"].astype(dt_np)
    wv = g["Wv"].astype(dt_np)
    wo = g["Wo"].astype(dt_np)
    fc1 = g["fc1_w"].astype(dt_np)
    fc2 = g["fc2_w"].astype(dt_np)
    hw1 = g["hw1"].astype(dt_np)
    hw2 = np.transpose(g["hw2"][:, :, 0].reshape(NOUT, DT, 128), (2, 0, 1))
    hw2 = np.ascontiguousarray(hw2, dtype=np.float32)
    embw = g["emb_w"].astype(np.float32)
    pe_full = g["pe"].astype(np.float32) + g["emb_b"][None, :].astype(np.float32)

    tril = (np.arange(128)[:, None] <= np.arange(128)[None, :])

    in_maps = []
    for c in range(NC):
        b, p = c // 4, c % 4
        blkA, blkB = p, 7 - p
        rows = np.r_[blkA * 128:(blkA + 1) * 128, blkB * 128:(blkB + 1) * 128]
        src_c = g["src"][b, rows, 0].astype(np.float32)        # [256]
        src_sb = np.ascontiguousarray(src_c.reshape(TH, 128).T)  # [128, TH]
        pe_c = pe_full[rows]                                    # [256, 1024]
        pe_sb = np.ascontiguousarray(
            np.transpose(pe_c.reshape(TH, 128, D), (1, 0, 2)))
        # masks [128 kv_p, NMB, 128 q], matching the attention ex layouts:
        #  chunk1 sub-pair sp: ex = [even: kv 2sp (A|B), kv 2sp+1 (A|B),
        #  odd: same] -> slots 8*sp + 4*rep + {A(2sp), 1, A(2sp+1), 1}
        #  chunk2: ex = [even: B(4..7), odd: same] -> slots 16 + 4*rep + j
        def a_mask(kb):
            if kb < blkA:
                return 1.0
            return tril if kb == blkA else 0.0

        def b_mask(kb):
            if kb < blkB:
                return 1.0
            return tril if kb == blkB else 0.0

        m = np.zeros((128, NMB, 128), dtype=dt_np)
        for sp in range(2):
            for rep in range(2):
                base = 8 * sp + 4 * rep
                m[:, base + 0, :] = a_mask(2 * sp)
                m[:, base + 1, :] = b_mask(2 * sp)
                m[:, base + 2, :] = a_mask(2 * sp + 1)
                m[:, base + 3, :] = b_mask(2 * sp + 1)
        for rep in range(2):
            for j in range(4):
                m[:, 16 + 4 * rep + j, :] = b_mask(4 + j)
        in_maps.append({
            "src": src_sb, "pe": pe_sb, "embw": embw,
            "masks": np.ascontiguousarray(m),
            "Wq": wq, "Wk": wk, "Wv": wv, "Wo": wo,
            "fc1w": fc1, "fc2w": fc2, "hw1": hw1, "hw2": hw2,
        })
    return in_maps


def _make_runner(nc):
    """Build the 8-core jitted PJRT callable once (same lowering path as
    run_bass_kernel_spmd under axon, but reusable across calls)."""
    import jax
    from jax.sharding import Mesh, PartitionSpec, NamedSharding
    from jax.experimental.shard_map import shard_map
    from concourse import bass2jax

    bass2jax.install_neuronx_cc_hook()
    partition_name = (nc.partition_id_tensor.name
                      if nc.partition_id_tensor else None)
    in_names, out_names, out_avals, zero_outs = [], [], [], []
    for alloc in nc.m.functions[0].allocations:
        if not isinstance(alloc, mybir.MemoryLocationSet):
            continue
        name = alloc.memorylocations[0].name
        if alloc.kind == "ExternalInput":
            if name != partition_name:
                in_names.append(name)
        elif alloc.kind == "ExternalOutput":
            out_names.append(name)
            shape = tuple(alloc.tensor_shape)
            dtype = mybir.dt.np(alloc.dtype)
            out_avals.append(jax.core.ShapedArray(shape, dtype))
            zero_outs.append(np.zeros(shape, dtype))
    all_in_names = list(in_names) + list(out_names)
    if partition_name is not None:
        all_in_names.append(partition_name)

    def _body(*args):
        operands = list(args)
        if partition_name is not None:
            operands.append(bass2jax.partition_id_tensor())
        outs = bass2jax._bass_exec_p.bind(
            *operands, out_avals=tuple(out_avals),
            in_names=tuple(all_in_names), out_names=tuple(out_names),
            lowering_input_output_aliases=(), sim_require_finite=True,
            sim_require_nnan=True, nc=nc)
        return tuple(outs)

    devices = jax.devices()[:NC]
    mesh = Mesh(np.asarray(devices), ("core",))
    n_args = len(in_names) + len(out_names)
    fn = jax.jit(shard_map(_body, mesh=mesh,
                           in_specs=(PartitionSpec("core"),) * n_args,
                           out_specs=(PartitionSpec("core"),) * len(out_names),
                           check_rep=False),
                 keep_unused=True)
    sharding = NamedSharding(mesh, PartitionSpec("core"))
    return fn, in_names, out_names, zero_outs, sharding


def _run_fast(nc, in_maps):
    """Execute with device-resident cached inputs; returns [T, NOUT] per core."""
    import jax
    import hashlib

    if "runner" not in _CACHE:
        _CACHE["runner"] = _make_runner(nc)
    fn, in_names, out_names, zero_outs, sharding = _CACHE["runner"]

    h = hashlib.sha1()
    for name in in_names:
        for c in range(NC):
            h.update(np.ascontiguousarray(in_maps[c][name]).tobytes())
    digest = h.hexdigest()
    if _CACHE.get("args_key") != digest:
        concat_in = [np.concatenate([np.asarray(in_maps[c][i])
                                     for c in range(NC)], axis=0)
                     for i in in_names]
        concat_zeros = [np.zeros((NC * z.shape[0], *z.shape[1:]), z.dtype)
                        for z in zero_outs]
        args = [jax.device_put(a, sharding) for a in concat_in + concat_zeros]
        jax.block_until_ready(args)
        _CACHE["args"] = args
        _CACHE["args_key"] = digest
    outs = fn(*_CACHE["args"])
    y = np.asarray(outs[out_names.index("y")])
    return y.reshape(NC, T, NOUT)


def kernel(**inputs) -> np.ndarray:
    dt_mm = mybir.dt.float16
    dt_np = np.float16
    key = ("prog", str(dt_mm))
    if key not in _CACHE:
        _CACHE[key] = _build(dt_mm)
    nc = _CACHE[key]
    in_maps = _prep_inputs(inputs, dt_np)
    try:
        per_core = _run_fast(nc, in_maps)
    except Exception:
        res = run_bass_kernel_spmd(nc, in_maps, core_ids=list(range(NC)))
        per_core = np.stack([res.results[c]["y"] for c in range(NC)])
    full = np.zeros((B, S, NOUT), dtype=np.float32)
    for c in range(NC):
        b, p = c // 4, c % 4
        blkA, blkB = p, 7 - p
        full[b, blkA * 128:(blkA + 1) * 128, :] = per_core[c][0:128]
        full[b, blkB * 128:(blkB + 1) * 128, :] = per_core[c][128:256]
    return full


if __name__ == "__main__":
    sys.path.insert(0, os.path.dirname(os.path.abspath(__file__)))
    import reference
    ins = reference.setup_inputs()
    want = np.asarray(reference.reference(**ins))
    got = kernel(**{k: np.asarray(v) for k, v in ins.items()})
    err = np.abs(got - want).max() / np.abs(want).max()
    print("Relative error:", err)
